# revision 9
# baseline (speedup 1.0000x reference)
"""Distributed Trainium2 Bass kernel for AMLGraphNet (2x GAT + 2x GCN + MLP head).

Sharding: nodes partitioned across 8 NeuronCores; edges grouped by
destination-node owner, sorted by dst, and packed into 128-edge chunks per
128-dst tile. Segment softmax / scatter-add run as one-hot matmuls on the
TensorEngine; source-node features are fetched with dma_gather from a
replicated (L1) or all-gathered (L2-L4) per-layer feature table in HBM.
"""
import numpy as np
import ml_dtypes

import concourse.bass as bass
import concourse.bacc as bacc
import concourse.tile as tile
import concourse.mybir as mybir
from concourse import bass_utils

BF = ml_dtypes.bfloat16
F32 = mybir.dt.float32
BF16 = mybir.dt.bfloat16
I16 = mybir.dt.int16
I32 = mybir.dt.int32
AF = mybir.ActivationFunctionType
OP = mybir.AluOpType
AX = mybir.AxisListType

P = 8                      # cores
HALF = 32768               # int16 gather index limit
BN_SCALE = 1.0 / np.sqrt(1.0 + 1e-5)
KB_MAX = 24                # max chunks per gather batch
PHASE = 7                  # debug: stop after phase N
GDBG = 3                   # gat_agg detail level (1=agg only, 2=+logits, 3=full)


def _bfc(x):
    return np.ascontiguousarray(np.asarray(x, dtype=np.float32).astype(BF))


def _f32c(x):
    return np.ascontiguousarray(np.asarray(x, dtype=np.float32))


def _rep(v, parts=128):
    v = np.asarray(v, np.float32).reshape(1, -1)
    return np.ascontiguousarray(np.broadcast_to(v, (parts, v.shape[1])))


def _ceil(a, b):
    return -(-a // b)


class Meta:
    pass


def _preprocess(inputs):
    x = np.asarray(inputs["x"], np.float32)
    ei = np.asarray(inputs["edge_index"], np.int64)
    N, F_IN = x.shape
    assert N % P == 0
    NL = N // P
    T = _ceil(NL, 128)            # local tiles per core
    NTG = _ceil(N, 128)           # global tiles
    NPAD = NTG * 128

    ar = np.arange(N, dtype=np.int64)
    src = np.concatenate([ei[0], ar]).astype(np.int64)
    dst = np.concatenate([ei[1], ar]).astype(np.int64)
    deg = np.bincount(dst, minlength=N).astype(np.float32)
    dinv = np.where(deg > 0, 1.0 / np.sqrt(deg), 0.0).astype(np.float32)
    wg = (dinv[src] * dinv[dst]).astype(np.float32)

    order = np.argsort(dst, kind="stable")
    src, dst, wg = src[order], dst[order], wg[order]

    m = Meta()
    m.N, m.F_IN, m.NL, m.T, m.NTG, m.NPAD = N, F_IN, NL, T, NTG, NPAD

    tile_edges = [[None] * T for _ in range(P)]
    Klo = np.zeros((P, T), np.int64)
    Khi = np.zeros((P, T), np.int64)
    for c in range(P):
        for t in range(T):
            lo_b = np.searchsorted(dst, c * NL + t * 128)
            hi_b = np.searchsorted(dst, min(c * NL + (t + 1) * 128, (c + 1) * NL))
            s_, d_, w_ = src[lo_b:hi_b], dst[lo_b:hi_b], wg[lo_b:hi_b]
            dl = (d_ - c * NL - t * 128).astype(np.int64)
            is_lo = s_ < HALF
            tile_edges[c][t] = (s_[is_lo], s_[~is_lo], dl[is_lo], dl[~is_lo],
                                w_[is_lo], w_[~is_lo])
            Klo[c, t] = _ceil(int(is_lo.sum()), 128)
            Khi[c, t] = _ceil(int((~is_lo).sum()), 128)
    KloM = Klo.max(0)
    KhiM = Khi.max(0)

    # batches of whole tiles, each <= KB_MAX chunks
    batches, cur, cnt = [], [], 0
    for t in range(T):
        kt = int(KloM[t] + KhiM[t])
        if cur and cnt + kt > KB_MAX:
            batches.append(cur)
            cur, cnt = [], 0
        cur.append(t)
        cnt += kt
    if cur:
        batches.append(cur)

    slot = 0
    m.batch_info = []
    for b in batches:
        nlo = int(sum(KloM[t] for t in b))
        nhi = int(sum(KhiM[t] for t in b))
        info = {"tiles": b, "k0": slot, "nlo": nlo, "nhi": nhi, "tl": {}}
        off = 0
        for t in b:
            info["tl"][t] = [off, int(KloM[t]), 0, int(KhiM[t])]
            off += int(KloM[t])
        for t in b:
            info["tl"][t][2] = off
            off += int(KhiM[t])
        m.batch_info.append(info)
        slot += nlo + nhi
    NCH = slot
    m.NCH = NCH

    idx16 = np.zeros((P, 128, NCH * 8), np.int16)
    oh = np.zeros((P, 128, NCH, 128), BF)
    ohT = np.zeros((P, 128, NCH, 128), BF)
    ohg = np.zeros((P, 128, NCH, 128), BF)
    one_bf = np.float32(1.0).astype(BF)
    for c in range(P):
        for info in m.batch_info:
            for t in info["tiles"]:
                lo0, nlo_t, hi0, nhi_t = info["tl"][t]
                slo, shi, dlo, dhi, wlo, whi = tile_edges[c][t]
                for (s_, d_, w_, s0, ns, bias) in (
                    (slo, dlo, wlo, info["k0"] + lo0, nlo_t, 0),
                    (shi, dhi, whi, info["k0"] + hi0, nhi_t, HALF),
                ):
                    if ns == 0:
                        continue
                    ne = len(s_)
                    e = np.arange(ne)
                    sl = s0 + e // 128
                    ep = e % 128
                    idxvals = np.zeros(ns * 128, np.int16)
                    idxvals[:ne] = (s_ - bias).astype(np.int16)
                    blk = idxvals.reshape(ns * 8, 16).T   # [16, ns*8]
                    cols = slice(s0 * 8, (s0 + ns) * 8)
                    for g in range(8):
                        idx16[c, g * 16:(g + 1) * 16, cols] = blk
                    oh[c, ep, sl, d_] = one_bf
                    ohT[c, d_, sl, ep] = one_bf
                    ohg[c, ep, sl, d_] = w_.astype(BF)

    # ---- weights (BN folded) ----
    def gat_w(W, a_s, a_d, b, g, be, heads, ch):
        s = np.asarray(g, np.float32) * BN_SCALE
        Weff = np.asarray(W, np.float32) * s[None, :]
        Wr = np.asarray(W, np.float32).reshape(W.shape[0], heads, ch)
        Wa_s = np.einsum("khc,hc->kh", Wr, np.asarray(a_s, np.float32))
        Wa_d = np.einsum("khc,hc->kh", Wr, np.asarray(a_d, np.float32))
        beff = np.asarray(b, np.float32) * s + np.asarray(be, np.float32)
        return Weff, Wa_s, Wa_d, beff

    W1e, Wa_s1, Wa_d1, b1e = gat_w(inputs["W1"], inputs["a_src1"], inputs["a_dst1"],
                                   inputs["b1"], inputs["g1"], inputs["be1"], 8, 64)
    W2e, Wa_s2, Wa_d2, b2e = gat_w(inputs["W2"], inputs["a_src2"], inputs["a_dst2"],
                                   inputs["b2"], inputs["g2"], inputs["be2"], 4, 64)
    s3 = np.asarray(inputs["g3"], np.float32) * BN_SCALE
    W3e = _f32c(inputs["W3"]) * s3[None, :]
    b3e = _f32c(inputs["b3"]) * s3 + _f32c(inputs["be3"])
    s4 = np.asarray(inputs["g4"], np.float32) * BN_SCALE
    W4e = _f32c(inputs["W4"]) * s4[None, :]
    b4e = _f32c(inputs["b4"]) * s4 + _f32c(inputs["be4"])

    cs1 = BN_SCALE * np.asarray(inputs["cg1"], np.float32)
    cbe1 = np.asarray(inputs["cbe1"], np.float32)
    cs2 = BN_SCALE * np.asarray(inputs["cg2"], np.float32)
    cbe2 = np.asarray(inputs["cbe2"], np.float32)
    cW1, cb1 = _f32c(inputs["cW1"]), _f32c(inputs["cb1"])
    cW2 = _f32c(inputs["cW2"]) * cs1[:, None]
    cb2 = _f32c(inputs["cb2"]) + cbe1 @ _f32c(inputs["cW2"])
    cW3 = _f32c(inputs["cW3"]) * cs2[:, None]
    cb3 = _f32c(inputs["cb3"]) + cbe2 @ _f32c(inputs["cW3"])
    cW4, cb4 = _f32c(inputs["cW4"]), _f32c(inputs["cb4"])

    m.F1 = W1e.shape[1]          # 512
    m.H1 = Wa_s1.shape[1]        # 8
    m.F2 = W2e.shape[1]          # 256
    m.H2 = Wa_s2.shape[1]        # 4
    m.F3 = W3e.shape[1]          # 64
    m.F4 = W4e.shape[1]          # 32
    m.CW1 = _ceil(m.F1 + 2 * m.H1, 128) * 128   # bf16 cols per table1 row

    common = {}
    xp = np.zeros((F_IN, NPAD), np.float32)
    xp[:, :N] = x.T
    common["xT"] = _bfc(xp)
    common["W1ext"] = _bfc(np.concatenate([W1e, Wa_s1, Wa_d1], axis=1))
    K2 = W2e.shape[0]
    m.K2T = K2 // 128
    w2x = np.concatenate([W2e, Wa_d2], axis=1)
    common["W2ext"] = _bfc(w2x.reshape(m.K2T, 128, w2x.shape[1]).transpose(1, 0, 2))
    K3 = W3e.shape[0]
    m.K3T = K3 // 128
    common["W3ext"] = _bfc(W3e.reshape(m.K3T, 128, m.F3).transpose(1, 0, 2))
    common["W4ext"] = _bfc(W4e)
    common["asrc2rep"] = _rep(np.asarray(inputs["a_src2"], np.float32).reshape(-1))
    common["b1rep"] = _rep(b1e)
    common["b2rep"] = _rep(b2e)
    common["b3rep"] = _rep(b3e)
    common["b4rep"] = _rep(b4e)
    common["cW1e"] = _bfc(cW1)
    common["cW2e"] = _bfc(cW2)
    common["cW3e"] = _bfc(cW3)
    common["cW4e"] = _bfc(cW4)
    common["cb1col"] = _f32c(cb1.reshape(-1, 1))
    common["cb2col"] = _f32c(cb2.reshape(-1, 1))
    common["cb3col"] = _f32c(cb3.reshape(-1, 1))
    common["cb4rep"] = _rep(cb4)

    per_core = []
    for c in range(P):
        d = dict(common)
        d["xTloc"] = _bfc(xp[:, c * NL:c * NL + T * 128])
        d["idx16"] = np.ascontiguousarray(idx16[c])
        d["ohgat"] = np.ascontiguousarray(oh[c])
        d["ohTgat"] = np.ascontiguousarray(ohT[c])
        d["ohgcn"] = np.ascontiguousarray(ohg[c])
        per_core.append(d)
    m.per_core = per_core
    return m


# ---------------------------------------------------------------------------


def _build(nc, tc, m):
    T, NCH, NPAD = m.T, m.NCH, m.NPAD
    F1, H1, F2, H2, F3, F4 = m.F1, m.H1, m.F2, m.H2, m.F3, m.F4
    CW1 = m.CW1
    NLP = T * 128

    def dram_in(name, shape, dt):
        return nc.dram_tensor(name, list(shape), dt, kind="ExternalInput")

    xT = dram_in("xT", (m.F_IN, NPAD), BF16)
    xTloc = dram_in("xTloc", (m.F_IN, NLP), BF16)
    W1ext = dram_in("W1ext", (m.F_IN, F1 + 2 * H1), BF16)
    W2ext = dram_in("W2ext", (128, m.K2T, F2 + H2), BF16)
    W3ext = dram_in("W3ext", (128, m.K3T, F3), BF16)
    W4ext = dram_in("W4ext", (F3, F4), BF16)
    asrc2rep = dram_in("asrc2rep", (128, F2), F32)
    b1rep = dram_in("b1rep", (128, F1), F32)
    b2rep = dram_in("b2rep", (128, F2), F32)
    b3rep = dram_in("b3rep", (128, F3), F32)
    b4rep = dram_in("b4rep", (128, F4), F32)
    cW1e = dram_in("cW1e", (F4, 32), BF16)
    cW2e = dram_in("cW2e", (32, 16), BF16)
    cW3e = dram_in("cW3e", (16, 8), BF16)
    cW4e = dram_in("cW4e", (8, 2), BF16)
    cb1col = dram_in("cb1col", (32, 1), F32)
    cb2col = dram_in("cb2col", (16, 1), F32)
    cb3col = dram_in("cb3col", (8, 1), F32)
    cb4rep = dram_in("cb4rep", (128, 2), F32)
    idx16 = dram_in("idx16", (128, NCH * 8), I16)
    ohgat = dram_in("ohgat", (128, NCH, 128), BF16)
    ohTgat = dram_in("ohTgat", (128, NCH, 128), BF16)
    ohgcn = dram_in("ohgcn", (128, NCH, 128), BF16)

    out = nc.dram_tensor("out", [m.NL, 2], F32, kind="ExternalOutput")

    table1 = nc.dram_tensor("table1", [NPAD, CW1], BF16, kind="Internal")
    z2shard = nc.dram_tensor("z2shard", [m.NL, F2], BF16, kind="Internal")
    table2 = nc.dram_tensor("table2", [m.N, F2], BF16, kind="Internal",
                            addr_space="Shared")
    shard3 = nc.dram_tensor("shard3", [m.NL, F3], F32, kind="Internal")
    table3 = nc.dram_tensor("table3", [m.N, F3], F32, kind="Internal",
                            addr_space="Shared")
    shard4 = nc.dram_tensor("shard4", [m.NL, 64], F32, kind="Internal")
    table4 = nc.dram_tensor("table4", [m.N, 64], F32, kind="Internal",
                            addr_space="Shared")
    h1T_d = nc.dram_tensor("h1T_d", [128, F1 // 128, NLP], BF16, kind="Internal")
    h2T_d = nc.dram_tensor("h2T_d", [128, F2 // 128, NLP], BF16, kind="Internal")
    h3T_d = nc.dram_tensor("h3T_d", [F3, NLP], BF16, kind="Internal")
    MLPC = _ceil(NLP, 512)
    h4T_d = nc.dram_tensor("h4T_d", [F4, MLPC * 512], BF16, kind="Internal")

    with tc.tile_pool(name="resident", bufs=1) as res:
        idx_sb = res.tile([128, NCH * 8], I16)
        nc.sync.dma_start(idx_sb[:], idx16[:])
        w1x_sb = res.tile([m.F_IN, F1 + 2 * H1], BF16)
        nc.sync.dma_start(w1x_sb[:], W1ext[:])
        w2x_sb = res.tile([128, m.K2T, F2 + H2], BF16)
        nc.sync.dma_start(w2x_sb[:], W2ext[:])
        w3x_sb = res.tile([128, m.K3T, F3], BF16)
        nc.sync.dma_start(w3x_sb[:], W3ext[:])
        w4x_sb = res.tile([F3, F4], BF16)
        nc.sync.dma_start(w4x_sb[:], W4ext[:])
        asrc2_sb = res.tile([128, F2], F32)
        nc.sync.dma_start(asrc2_sb[:], asrc2rep[:])
        b1_sb = res.tile([128, F1], F32)
        nc.sync.dma_start(b1_sb[:], b1rep[:])
        b2_sb = res.tile([128, F2], F32)
        nc.sync.dma_start(b2_sb[:], b2rep[:])
        b3_sb = res.tile([128, F3], F32)
        nc.sync.dma_start(b3_sb[:], b3rep[:])
        b4_sb = res.tile([128, F4], F32)
        nc.sync.dma_start(b4_sb[:], b4rep[:])

        # f32 identity for PE transposes
        iota_r = res.tile([128, 128], I32)
        nc.gpsimd.iota(iota_r[:], pattern=[[0, 128]], base=0, channel_multiplier=1)
        iota_c = res.tile([128, 128], I32)
        nc.gpsimd.iota(iota_c[:], pattern=[[1, 128]], base=0, channel_multiplier=0)
        ident = res.tile([128, 128], F32)
        nc.vector.tensor_tensor(ident[:], iota_r[:], iota_c[:], OP.is_equal)

        al_d1 = res.tile([128, T * H1], BF16)
        al_d2 = res.tile([128, T * H2], BF16)

        # ============ L1 z: replicated table1 build ============
        SLAB = 16
        with (
            tc.tile_pool(name="l1z_sb", bufs=2) as sbp,
            tc.tile_pool(name="l1z_ps", bufs=2, space="PSUM") as psp,
        ):
            for sl in range(_ceil(m.NTG, SLAB)):
                t0 = sl * SLAB
                nt = min(SLAB, m.NTG - t0)
                xsl = sbp.tile([128, SLAB * 128], BF16, tag="xsl")
                nc.sync.dma_start(xsl[:, :nt * 128], xT[:, t0 * 128:(t0 + nt) * 128])
                tab = sbp.tile([128, SLAB, CW1], BF16, tag="tab")
                tabf = tab[:].bitcast(F32)
                if CW1 > F1 + 2 * H1:
                    nc.vector.memset(tab[:, :, F1 + 2 * H1:], 0.0)
                for j in range(nt):
                    zp = psp.tile([128, F1 + 2 * H1], F32, tag="zp")
                    lhs = xsl[:, j * 128:(j + 1) * 128]
                    nc.tensor.matmul(zp[:, :F1], lhs, w1x_sb[:, :F1],
                                     start=True, stop=True)
                    nc.tensor.matmul(zp[:, F1:], lhs, w1x_sb[:, F1:],
                                     start=True, stop=True)
                    nc.vector.tensor_copy(tab[:, j, :F1], zp[:, :F1])
                    nc.vector.tensor_copy(tabf[:, j, F1 // 2:F1 // 2 + H1],
                                          zp[:, F1:F1 + H1])
                nc.sync.dma_start(
                    table1[:].rearrange("(a p) c -> p a c", p=128)[:, t0:t0 + nt, :],
                    tab[:, :nt, :])
            for t in range(T):
                adp = psp.tile([128, H1], F32, tag="adp")
                xls = sbp.tile([128, 128], BF16, tag="xls")
                nc.sync.dma_start(xls[:], xTloc[:, t * 128:(t + 1) * 128])
                nc.tensor.matmul(adp[:], xls[:], w1x_sb[:, F1 + H1:],
                                 start=True, stop=True)
                nc.vector.tensor_copy(al_d1[:, t * H1:(t + 1) * H1], adp[:])


        GMAX = 6   # max 128-idx chunks per dma_gather (SWDGE ring capacity)

        def gather_split(dst_tile, src_ap, k0_dst, k0_idx, nchunks, cw):
            o = 0
            while o < nchunks:
                n = min(GMAX, nchunks - o)
                nc.gpsimd.dma_gather(
                    dst_tile[:, k0_dst + o:k0_dst + o + n, :], src_ap,
                    idx_sb[:, (k0_idx + o) * 8:(k0_idx + o + n) * 8],
                    n * 128, n * 128, cw)
                o += n

        # ============ shared GAT aggregation ============
        def gat_agg(table, tbl_rows, CW, F, H, al_d_res, als_inline, bias_sb,
                    hT_d, hT_K):
            with (
                tc.tile_pool(name="ga_sb", bufs=2) as sbp,
                tc.tile_pool(name="ga_ht", bufs=2) as htp,
                tc.tile_pool(name="ga_ps", bufs=2, space="PSUM") as psp,
                tc.tile_pool(name="ga_ps2", bufs=2, space="PSUM") as psp2,
            ):
                for info in m.batch_info:
                    k0, nlo, nhi = info["k0"], info["nlo"], info["nhi"]
                    kb = nlo + nhi
                    g_t = sbp.tile([128, KB_MAX, CW], BF16, tag="g")
                    gf = g_t[:].bitcast(F32)
                    if nlo:
                        gather_split(g_t, table[:tbl_rows, :], 0, k0, nlo, CW)
                    if nhi:
                        gather_split(g_t, table[HALF:tbl_rows, :], nlo, k0 + nlo,
                                     nhi, CW)
                    ohs = sbp.tile([128, KB_MAX, 128], BF16, tag="ohs")
                    nc.sync.dma_start(ohs[:, :kb, :], ohgat[:, k0:k0 + kb, :])
                    ohTs = sbp.tile([128, KB_MAX, 128], BF16, tag="ohTs")
                    nc.sync.dma_start(ohTs[:, :kb, :], ohTgat[:, k0:k0 + kb, :])
                    ntl = len(info["tiles"])
                    hts = htp.tile([128, hT_K, ntl * 128], BF16, tag="hts")

                    for ti, t in enumerate(info["tiles"]):
                        lo0, nlo_t, hi0, nhi_t = info["tl"][t]
                        kt = nlo_t + nhi_t
                        if kt == 0:
                            continue
                        slots = (list(range(lo0, lo0 + nlo_t))
                                 + list(range(hi0, hi0 + nhi_t)))
                        if GDBG < 2:
                            if GDBG < 1:
                                nc.sync.dma_start(
                                    g_t[:, :kt, :],
                                    table[:].rearrange("(a p) c -> p a c", p=128)[:, :kt, :])
                            agg = psp.tile([128, F], F32, tag="agg")
                            for kk, s in enumerate(slots):
                                nc.tensor.matmul(agg[:], ohs[:, s, :],
                                                 g_t[:, s, :F], start=(kk == 0),
                                                 stop=(kk == kt - 1))
                            hsb = sbp.tile([128, F], F32, tag="hsb")
                            nc.vector.tensor_copy(hsb[:], agg[:])
                            for q in range(hT_K):
                                tp = psp.tile([128, 128], F32, tag="tp")
                                nc.tensor.transpose(tp[:], hsb[:, q * 128:(q + 1) * 128],
                                                    ident[:])
                                nc.vector.tensor_copy(hts[:, q, ti * 128:(ti + 1) * 128],
                                                      tp[:])
                            continue
                        alde = psp2.tile([128, kt, H], F32, tag="alde")
                        for kk, s in enumerate(slots):
                            nc.tensor.matmul(
                                alde[:, kk, :], ohTs[:, s, :],
                                al_d_res[:, t * H:(t + 1) * H],
                                start=True, stop=True)
                        lg = sbp.tile([128, kt, H], F32, tag="lg")
                        for gi, (o0, nk) in enumerate(((lo0, nlo_t), (hi0, nhi_t))):
                            if nk == 0:
                                continue
                            kk0 = 0 if gi == 0 else nlo_t
                            if als_inline:
                                als_ap = gf[:, o0:o0 + nk, F // 2:F // 2 + H]
                            else:
                                tmp = sbp.tile([128, kt, F], F32, tag="alstmp")
                                nc.vector.tensor_tensor(
                                    tmp[:, kk0:kk0 + nk, :],
                                    g_t[:, o0:o0 + nk, :F],
                                    asrc2_sb[:].unsqueeze(1).to_broadcast(
                                        (128, nk, F)),
                                    OP.mult)
                                als_sb = sbp.tile([128, kt, H], F32, tag="alssb")
                                nc.vector.tensor_reduce(
                                    als_sb[:, kk0:kk0 + nk, :],
                                    tmp[:, kk0:kk0 + nk, :].rearrange(
                                        "p k (h c) -> p k h c", h=H),
                                    AX.X, OP.add)
                                als_ap = als_sb[:, kk0:kk0 + nk, :]
                            nc.vector.tensor_tensor(
                                lg[:, kk0:kk0 + nk, :], als_ap,
                                alde[:, kk0:kk0 + nk, :], OP.add)
                        lg2 = sbp.tile([128, kt, H], F32, tag="lg2")
                        nc.vector.scalar_tensor_tensor(lg2[:], lg[:, :kt, :], 0.2,
                                                       lg[:, :kt, :], OP.mult, OP.max)
                        ex = sbp.tile([128, kt, H], BF16, tag="ex")
                        nc.scalar.activation(ex[:], lg2[:], AF.Exp)
                        if GDBG < 3:
                            agg = psp.tile([128, F], F32, tag="agg")
                            for kk, s in enumerate(slots):
                                nc.tensor.matmul(agg[:], ohs[:, s, :],
                                                 g_t[:, s, :F], start=(kk == 0),
                                                 stop=(kk == kt - 1))
                            hsb = sbp.tile([128, F], F32, tag="hsb")
                            nc.vector.tensor_tensor(
                                hsb[:], agg[:],
                                ex[:, 0, :].unsqueeze(2).to_broadcast((128, H, F // H)).rearrange("p h c -> p (h c)") if False else agg[:],
                                OP.add)
                            for q in range(hT_K):
                                tp = psp.tile([128, 128], F32, tag="tp")
                                nc.tensor.transpose(tp[:], hsb[:, q * 128:(q + 1) * 128],
                                                    ident[:])
                                nc.vector.tensor_copy(hts[:, q, ti * 128:(ti + 1) * 128],
                                                      tp[:])
                            continue
                        msg = sbp.tile([128, kt, F], BF16, tag="msg")
                        for gi, (o0, nk) in enumerate(((lo0, nlo_t), (hi0, nhi_t))):
                            if nk == 0:
                                continue
                            kk0 = 0 if gi == 0 else nlo_t
                            nc.vector.tensor_tensor(
                                msg[:, kk0:kk0 + nk, :].rearrange(
                                    "p k (h c) -> p k h c", h=H),
                                g_t[:, o0:o0 + nk, :F].rearrange(
                                    "p k (h c) -> p k h c", h=H),
                                ex[:, kk0:kk0 + nk, :].unsqueeze(3).to_broadcast(
                                    (128, nk, H, F // H)),
                                OP.mult)
                        agg = psp.tile([128, F], F32, tag="agg")
                        den = psp2.tile([128, H], F32, tag="den")
                        for kk, s in enumerate(slots):
                            nc.tensor.matmul(agg[:], ohs[:, s, :], msg[:, kk, :],
                                             start=(kk == 0), stop=(kk == kt - 1))
                            nc.tensor.matmul(den[:], ohs[:, s, :], ex[:, kk, :],
                                             start=(kk == 0), stop=(kk == kt - 1))
                        rden = sbp.tile([128, H], F32, tag="rden")
                        nc.vector.tensor_scalar_add(rden[:], den[:], 1e-30)
                        rden2 = sbp.tile([128, H], F32, tag="rden2")
                        nc.vector.reciprocal(rden2[:], rden[:])
                        hsb = sbp.tile([128, F], F32, tag="hsb")
                        nc.vector.tensor_tensor(
                            hsb[:].rearrange("p (h c) -> p h c", h=H),
                            agg[:].rearrange("p (h c) -> p h c", h=H),
                            rden2[:].unsqueeze(2).to_broadcast((128, H, F // H)),
                            OP.mult)
                        nc.vector.tensor_tensor(hsb[:], hsb[:], bias_sb[:], OP.add)
                        esb = sbp.tile([128, F], F32, tag="esb")
                        nc.scalar.activation(esb[:], hsb[:], AF.Exp)
                        nc.vector.tensor_scalar_max(hsb[:], hsb[:], 0.0)
                        nc.vector.scalar_tensor_tensor(hsb[:], esb[:], -1.0, hsb[:],
                                                       OP.add, OP.min)
                        for q in range(hT_K):
                            tp = psp.tile([128, 128], F32, tag="tp")
                            nc.tensor.transpose(tp[:], hsb[:, q * 128:(q + 1) * 128],
                                                ident[:])
                            nc.vector.tensor_copy(hts[:, q, ti * 128:(ti + 1) * 128],
                                                  tp[:])
                    t0 = info["tiles"][0]
                    nc.sync.dma_start(hT_d[:, :, t0 * 128:(t0 + ntl) * 128],
                                      hts[:, :, :ntl * 128])

        if PHASE >= 2:
            gat_agg(table1, NPAD, CW1, F1, H1, al_d1, True, b1_sb, h1T_d, F1 // 128)

        # ============ L2 z: local z2 + AllGather ============
        if PHASE < 3:
            with tc.tile_pool(name="dummy", bufs=1) as dp:
                d = dp.tile([128, 2], F32)
                nc.vector.memset(d[:], 0.0)
                TFd = m.NL // 128
                if TFd:
                    nc.sync.dma_start(
                        out[:TFd * 128, :].rearrange("(t p) c -> p t c", p=128),
                        d[:].unsqueeze(1).to_broadcast((128, TFd, 2)))
                if m.NL % 128:
                    nc.sync.dma_start(out[TFd * 128:, :], d[:m.NL % 128, :])
            return
        with (
            tc.tile_pool(name="l2z_sb", bufs=2) as sbp,
            tc.tile_pool(name="l2z_ps", bufs=2, space="PSUM") as psp,
        ):
            NSL = 4
            for sl in range(_ceil(T, NSL)):
                t0 = sl * NSL
                nt = min(NSL, T - t0)
                hsl = sbp.tile([128, F1 // 128, NSL * 128], BF16, tag="hsl")
                nc.sync.dma_start(hsl[:, :, :nt * 128],
                                  h1T_d[:, :, t0 * 128:(t0 + nt) * 128])
                for j in range(nt):
                    t = t0 + j
                    zp = psp.tile([128, F2 + H2], F32, tag="zp2")
                    for q in range(m.K2T):
                        nc.tensor.matmul(zp[:], hsl[:, q, j * 128:(j + 1) * 128],
                                         w2x_sb[:, q, :], start=(q == 0),
                                         stop=(q == m.K2T - 1))
                    zsb = sbp.tile([128, F2], BF16, tag="zsb")
                    nc.vector.tensor_copy(zsb[:], zp[:, :F2])
                    nc.vector.tensor_copy(al_d2[:, t * H2:(t + 1) * H2], zp[:, F2:])
                    nr = min(128, m.NL - t * 128)
                    nc.sync.dma_start(z2shard[t * 128:t * 128 + nr, :], zsb[:nr, :])
            nc.gpsimd.collective_compute(
                "AllGather", OP.bypass, replica_groups=[list(range(P))],
                ins=[z2shard[:]], outs=[table2[:]])

        if PHASE < 4:
            with tc.tile_pool(name="dummy", bufs=1) as dp:
                d = dp.tile([128, 2], F32)
                nc.vector.memset(d[:], 0.0)
                TFd = m.NL // 128
                if TFd:
                    nc.sync.dma_start(
                        out[:TFd * 128, :].rearrange("(t p) c -> p t c", p=128),
                        d[:].unsqueeze(1).to_broadcast((128, TFd, 2)))
                if m.NL % 128:
                    nc.sync.dma_start(out[TFd * 128:, :], d[:m.NL % 128, :])
            return
        gat_agg(table2, m.N, F2, F2, H2, al_d2, False, b2_sb, h2T_d, F2 // 128)

        # ============ GCN layers ============
        def gcn_z(hT_dram, hT_K, w_sb, KT, Fo, shard, tableo, zpad):
            with (
                tc.tile_pool(name="gz_sb", bufs=2) as sbp,
                tc.tile_pool(name="gz_ps", bufs=2, space="PSUM") as psp,
            ):
                NSL = 4
                for sl in range(_ceil(T, NSL)):
                    t0 = sl * NSL
                    nt = min(NSL, T - t0)
                    if hT_K > 1:
                        hsl = sbp.tile([128, hT_K, NSL * 128], BF16, tag="ghsl")
                        nc.sync.dma_start(hsl[:, :, :nt * 128],
                                          hT_dram[:, :, t0 * 128:(t0 + nt) * 128])
                    else:
                        hsl = sbp.tile([F3, 1, NSL * 128], BF16, tag="ghsl")
                        nc.sync.dma_start(hsl[:, 0, :nt * 128],
                                          hT_dram[:F3, t0 * 128:(t0 + nt) * 128])
                    for j in range(nt):
                        t = t0 + j
                        zp = psp.tile([128, Fo], F32, tag="gzp")
                        for q in range(KT):
                            lhs = (hsl[:, q, j * 128:(j + 1) * 128] if hT_K > 1
                                   else hsl[:F3, 0, j * 128:(j + 1) * 128])
                            rhs = w_sb[:, q, :] if KT > 1 else w_sb[:]
                            nc.tensor.matmul(zp[:], lhs, rhs, start=(q == 0),
                                             stop=(q == KT - 1))
                        zsb = sbp.tile([128, zpad], F32, tag="gzsb")
                        nc.vector.tensor_copy(zsb[:, :Fo], zp[:])
                        if zpad > Fo:
                            nc.vector.memset(zsb[:, Fo:], 0.0)
                        nr = min(128, m.NL - t * 128)
                        nc.sync.dma_start(shard[t * 128:t * 128 + nr, :],
                                          zsb[:nr, :])
                nc.gpsimd.collective_compute(
                    "AllGather", OP.bypass, replica_groups=[list(range(P))],
                    ins=[shard[:]], outs=[tableo[:]])

        def gcn_agg(table, CWf, Fo, bias_sb, hT_dram, hT_rows):
            with (
                tc.tile_pool(name="gc_sb", bufs=2) as sbp,
                tc.tile_pool(name="gc_ht", bufs=2) as htp,
                tc.tile_pool(name="gc_ps", bufs=2, space="PSUM") as psp,
            ):
                for info in m.batch_info:
                    k0, nlo, nhi = info["k0"], info["nlo"], info["nhi"]
                    kb = nlo + nhi
                    g_t = sbp.tile([128, KB_MAX, CWf], F32, tag="gg")
                    if nlo:
                        gather_split(g_t, table[:m.N, :], 0, k0, nlo, CWf)
                    if nhi:
                        gather_split(g_t, table[HALF:m.N, :], nlo, k0 + nlo,
                                     nhi, CWf)
                    ohs = sbp.tile([128, KB_MAX, 128], BF16, tag="gohs")
                    nc.sync.dma_start(ohs[:, :kb, :], ohgcn[:, k0:k0 + kb, :])
                    gb = sbp.tile([128, KB_MAX, Fo], BF16, tag="gb")
                    nc.vector.tensor_copy(gb[:, :kb, :], g_t[:, :kb, :Fo])
                    ntl = len(info["tiles"])
                    hts = htp.tile([128, ntl * 128], BF16, tag="ghts")
                    for ti, t in enumerate(info["tiles"]):
                        lo0, nlo_t, hi0, nhi_t = info["tl"][t]
                        kt = nlo_t + nhi_t
                        if kt == 0:
                            continue
                        slots = (list(range(lo0, lo0 + nlo_t))
                                 + list(range(hi0, hi0 + nhi_t)))
                        agg = psp.tile([128, Fo], F32, tag="gagg")
                        for kk, s in enumerate(slots):
                            nc.tensor.matmul(agg[:], ohs[:, s, :], gb[:, s, :],
                                             start=(kk == 0), stop=(kk == kt - 1))
                        hsb = sbp.tile([128, Fo], F32, tag="ghsb")
                        nc.vector.tensor_tensor(hsb[:], agg[:], bias_sb[:, :Fo],
                                                OP.add)
                        esb = sbp.tile([128, Fo], F32, tag="gesb")
                        nc.scalar.activation(esb[:], hsb[:], AF.Exp)
                        nc.vector.tensor_scalar_max(hsb[:], hsb[:], 0.0)
                        nc.vector.scalar_tensor_tensor(hsb[:], esb[:], -1.0, hsb[:],
                                                       OP.add, OP.min)
                        tp = psp.tile([128, 128], F32, tag="gtp")
                        nc.tensor.transpose(tp[:Fo, :], hsb[:], ident[:])
                        nc.vector.tensor_copy(hts[:Fo, ti * 128:(ti + 1) * 128],
                                              tp[:Fo, :])
                    t0 = info["tiles"][0]
                    nc.sync.dma_start(hT_dram[:hT_rows, t0 * 128:(t0 + ntl) * 128],
                                      hts[:hT_rows, :ntl * 128])

        gcn_z(h2T_d, F2 // 128, w3x_sb, m.K3T, F3, shard3, table3, F3)
        gcn_agg(table3, F3, F3, b3_sb, h3T_d, F3)
        gcn_z(h3T_d, 1, w4x_sb, 1, F4, shard4, table4, 64)
        gcn_agg(table4, 64, F4, b4_sb, h4T_d, F4)

        if MLPC * 512 > NLP:
            with tc.tile_pool(name="ztail", bufs=1) as zp_:
                zt = zp_.tile([F4, MLPC * 512 - NLP], BF16)
                nc.vector.memset(zt[:], 0.0)
                nc.sync.dma_start(h4T_d[:, NLP:], zt[:])

        # ============ MLP head + log_softmax ============
        with (
            tc.tile_pool(name="mlp_sb", bufs=2) as sbp,
            tc.tile_pool(name="mlp_res", bufs=1) as rp,
            tc.tile_pool(name="mlp_ps", bufs=2, space="PSUM") as psp,
        ):
            cw1 = rp.tile([F4, 32], BF16)
            nc.sync.dma_start(cw1[:], cW1e[:])
            cw2 = rp.tile([32, 16], BF16)
            nc.sync.dma_start(cw2[:], cW2e[:])
            cw3 = rp.tile([16, 8], BF16)
            nc.sync.dma_start(cw3[:], cW3e[:])
            cw4 = rp.tile([8, 2], BF16)
            nc.sync.dma_start(cw4[:], cW4e[:])
            cb1 = rp.tile([32, 1], F32)
            nc.sync.dma_start(cb1[:], cb1col[:])
            cb2 = rp.tile([16, 1], F32)
            nc.sync.dma_start(cb2[:], cb2col[:])
            cb3 = rp.tile([8, 1], F32)
            nc.sync.dma_start(cb3[:], cb3col[:])
            cb4 = rp.tile([128, 2], F32)
            nc.sync.dma_start(cb4[:], cb4rep[:])
            lgall = rp.tile([128, T, 2], F32)
            for ch in range(MLPC):
                n0 = ch * 512
                h4s = sbp.tile([F4, 512], BF16, tag="h4s")
                nc.sync.dma_start(h4s[:], h4T_d[:, n0:n0 + 512])
                p1 = psp.tile([32, 512], F32, tag="p1")
                nc.tensor.matmul(p1[:], cw1[:], h4s[:], start=True, stop=True)
                s1 = sbp.tile([32, 512], BF16, tag="s1")
                nc.scalar.activation(s1[:], p1[:], AF.Relu, bias=cb1[:])
                p2 = psp.tile([16, 512], F32, tag="p2")
                nc.tensor.matmul(p2[:], cw2[:], s1[:], start=True, stop=True)
                s2 = sbp.tile([16, 512], BF16, tag="s2")
                nc.scalar.activation(s2[:], p2[:], AF.Relu, bias=cb2[:])
                p3 = psp.tile([8, 512], F32, tag="p3")
                nc.tensor.matmul(p3[:], cw3[:], s2[:], start=True, stop=True)
                s3 = sbp.tile([8, 512], BF16, tag="s3")
                nc.scalar.activation(s3[:], p3[:], AF.Relu, bias=cb3[:])
                for j4 in range(4):
                    t = ch * 4 + j4
                    if t >= T:
                        break
                    p4 = psp.tile([128, 2], F32, tag="p4")
                    nc.tensor.matmul(p4[:], s3[:, j4 * 128:(j4 + 1) * 128], cw4[:],
                                     start=True, stop=True)
                    nc.vector.tensor_tensor(lgall[:, t, :], p4[:], cb4[:], OP.add)
            ex2 = rp.tile([128, T, 2], F32)
            nc.scalar.activation(ex2[:], lgall[:], AF.Exp)
            se = rp.tile([128, T], F32)
            nc.vector.tensor_reduce(se[:], ex2[:], AX.X, OP.add)
            nc.scalar.activation(se[:], se[:], AF.Ln)
            outsb = rp.tile([128, T, 2], F32)
            nc.vector.tensor_tensor(
                outsb[:], lgall[:],
                se[:].unsqueeze(2).to_broadcast((128, T, 2)), OP.subtract)
            TF = m.NL // 128
            nc.sync.dma_start(
                out[:TF * 128, :].rearrange("(t p) c -> p t c", p=128),
                outsb[:, :TF, :])
            if m.NL % 128:
                nc.sync.dma_start(out[TF * 128:, :], outsb[:m.NL % 128, TF, :])


def _run(m, trace=False):
    nc = bacc.Bacc("TRN2", target_bir_lowering=False, debug=False, num_devices=P)
    with tile.TileContext(nc) as tc:
        _build(nc, tc, m)
    nc.compile()
    res = bass_utils.run_bass_kernel_spmd(nc, m.per_core, core_ids=list(range(P)),
                                          trace=trace)
    out = np.concatenate([r["out"] for r in res.results], axis=0)
    return out, res


def kernel(**inputs):
    m = _preprocess(inputs)
    out, _ = _run(m)
    return out
